# revision 4
# baseline (speedup 1.0000x reference)
"""TextCNN-style conv layer (kernel sizes 3/4/5, EMB=300 -> DEPTH=256, bias,
ReLU, max-pool over time) as a Bass/Tile kernel for 8 Trainium2 NeuronCores.

Strategy: data-parallel over batch (8 samples per core), weights replicated.

Conv as dense-K matmuls over a host-materialized im2col: for branch n, window
output y[d, i] = sum_{k < n*300} Xrep[k, i] * Wn[d, k] with Xrep[k, i] =
x[i + k//300, k%300].  Each branch contracts ceil(n*300/128) dense K=128
tiles (8/10/12 -> 30 matmuls per sample per depth-half); branch boundaries
inside a tile are handled by zero-padding the *weights*.  fp16 operands run
the PE at its full 1 col/cycle rate; warm back-to-back gap is N/2.4GHz+2.5ns,
so the 480-matmul stream floor is ~79.4us.

v2 changes (from trace analysis of the 102.5us baseline):
- DMA: x ships as 4 chunks per sample of [128, 3, 400] fp16 with the DRAM
  laid out partition-major so every chunk is one 307KB transfer with 2.4KB
  contiguous per partition (the old per-K-tile loads were 102KB with 800B
  runs -> only ~230GB/s aggregate and a 2.9us weight stall mid-stream).
  All transfers are issued up-front in global need order, greedily
  byte-balanced across the two HWDGE queues (sync/scalar), which share the
  16 SDMA engines round-robin.
- PE warmup: the HAM clock gate starts every kernel at 1.2GHz and only
  reaches 2.4GHz after ~3.4us of sustained PE activity; the baseline paid
  that on its first 18 real matmuls (~2.9us).  Six dummy N=512 matmuls on a
  memset tile run during the DMA fill window instead.
- Epilogue: bias-add + relu fused per sample right after its reduces, so the
  tail after the last matmul is just one reduce + one tiny DVE op + the 6KB
  output DMA.

Epilogue math: relu(max_i(y + b)) == max(0, max_i y + b): DVE reduce_max over
the window axis straight out of PSUM, output staged [d, branch, half, sample]
per core and de-transposed on host.
"""

import numpy as np

B, SEQ, EMB = 64, 394, 300
DEPTH = 256
NCORES = 8
BPC = B // NCORES  # samples per core
SEQP = 400  # x_t free-dim padded (zeros) so shifted loads stay in bounds
NS = (3, 4, 5)
NTILES = (8, 10, 12)  # ceil(n*300/128) K-tiles per branch
COLB = (0, 8, 18)  # weight column base per branch
NCOL = 30
KTOT = 12  # distinct Xrep K-tiles per sample
NCHUNK = 4  # x DMA granularity: 4 chunks of 3 K-tiles per sample

TRACE = False
LAST_RESULT = None

_built = None


def _build_bass():
    import concourse.mybir as mybir
    import concourse.tile as tile
    from concourse import bacc
    from contextlib import ExitStack

    f32 = mybir.dt.float32
    f16 = mybir.dt.float16

    nc = bacc.Bacc("TRN2", target_bir_lowering=False)
    xt_d = nc.dram_tensor("xt", (BPC, NCHUNK, 128, 3, SEQP), f16, kind="ExternalInput")
    wq_d = nc.dram_tensor("wq", (128, 2, NCOL, 128), f16, kind="ExternalInput")
    bp_d = nc.dram_tensor("bp", (128, 3, 2), f32, kind="ExternalInput")
    out_d = nc.dram_tensor("out_t", (128, 3, 2, BPC), f32, kind="ExternalOutput")

    with tile.TileContext(nc) as tc, ExitStack() as ctx:
        xpool = ctx.enter_context(tc.tile_pool(name="x", bufs=1))
        wpool = ctx.enter_context(tc.tile_pool(name="w", bufs=1))
        cpool = ctx.enter_context(tc.tile_pool(name="consts", bufs=1))
        spool = ctx.enter_context(tc.tile_pool(name="stage", bufs=1))
        pspool = ctx.enter_context(tc.tile_pool(name="ps", bufs=8, space="PSUM"))

        # --- PE warmup: six dummy N=512 matmuls on memset tiles so the HAM
        # clock gate reaches 2.4GHz while the first DMAs are still in flight.
        wz = cpool.tile([128, 128], f16)
        xz = cpool.tile([128, 512], f16)
        dmy = cpool.tile([128, 1], f32)
        nc.vector.memset(wz[:], 0.0)
        nc.vector.memset(xz[:], 0.0)
        psw = pspool.tile([128, 512], f32, tag="ps", name="ps_warm")
        NWARM = 6
        for i in range(NWARM):
            nc.tensor.matmul(
                psw[:], lhsT=wz[:], rhs=xz[:], start=(i == 0), stop=(i == NWARM - 1)
            )
        nc.vector.reduce_max(dmy[:], psw[:], axis=mybir.AxisListType.X)

        # --- SBUF destination tiles
        wts = {}
        for dh in range(2):
            for br in range(3):
                wts[dh, br] = wpool.tile(
                    [128, NTILES[br], 128], f16, tag=f"w{dh}{br}", name=f"w{dh}{br}"
                )
        xs = {}
        for s in range(BPC):
            for c in range(NCHUNK):
                xs[s, c] = xpool.tile(
                    [128, 3, SEQP], f16, tag=f"x{s}c{c}", name=f"x{s}c{c}"
                )
        bt = cpool.tile([128, 3, 2], f32)

        # --- All loads issued up-front in global need order, greedily
        # byte-balanced across the two HWDGE queues (they share the SDMA
        # engines at packet granularity, so each drains ~half the bytes).
        def _w_item(dh, br):
            nt = NTILES[br]
            return (nt * 256 * 128, wts[dh, br][:], wq_d[:, dh, COLB[br] : COLB[br] + nt, :])

        def _x_item(s, c):
            return (3 * SEQP * 2 * 128, xs[s, c][:], xt_d[s, c])

        # Need-ordered load list.  Sample 0's first chunk is split so the very
        # first matmul only waits on one 102KB transfer (subtile deps); the
        # tiny bias transfer is demoted behind sample 1 (it is only needed by
        # the first epilogue, ~10us into the stream) so its slow small-
        # descriptor path cannot delay the head of a queue.
        items = [
            _w_item(0, 0),
            (SEQP * 2 * 128, xs[0, 0][:, 0:1, :], xt_d[0, 0, :, 0:1, :]),
            (2 * SEQP * 2 * 128, xs[0, 0][:, 1:3, :], xt_d[0, 0, :, 1:3, :]),
        ]
        items += [_x_item(0, c) for c in (1, 2)]
        items += [_w_item(0, 1), _x_item(0, 3), _w_item(0, 2)]
        items += [_w_item(1, 0), _w_item(1, 1), _w_item(1, 2)]
        items += [_x_item(1, c) for c in range(NCHUNK)]
        items += [(3 * 2 * 4 * 128, bt[:], bp_d[:])]
        for s in range(2, BPC):
            items += [_x_item(s, c) for c in range(NCHUNK)]

        qbytes = [0, 0]
        qeng = (nc.sync, nc.scalar)
        for nbytes, dst, src in items:
            q = 0 if qbytes[0] <= qbytes[1] else 1
            qeng[q].dma_start(dst, src)
            qbytes[q] += nbytes

        stage = spool.tile([128, 3, 2, BPC], f32)
        stage2 = spool.tile([128, 3, 2, BPC], f32)

        def do_group(s, dh, br):
            n = NS[br]
            nw = SEQ - n  # windows the reference maxes over
            nmm = nw + (nw & 1)  # keep the moving free-dim count even
            nt = NTILES[br]
            ps = pspool.tile([128, 512], f32, tag="ps", name=f"ps_{s}_{dh}_{br}")
            for r in range(nt):
                kk = min(128, 5 * EMB - 128 * r)  # 92 on the last tile
                nc.tensor.matmul(
                    ps[:, :nmm],
                    lhsT=wts[dh, br][:kk, r, :],
                    rhs=xs[s, r // 3][:kk, r % 3, :nmm],
                    start=(r == 0),
                    stop=(r == nt - 1),
                )
            nc.vector.reduce_max(
                stage[:, br, dh, s : s + 1],
                ps[:, :nw],
                axis=mybir.AxisListType.X,
            )

        for s in range(BPC):
            for dh in range(2):
                for br in range(3):
                    do_group(s, dh, br)
            # fused bias + relu for this sample's column, off the tail
            nc.vector.tensor_tensor(
                stage2[:, :, :, s : s + 1],
                stage[:, :, :, s : s + 1],
                bt[:, :, :, None].to_broadcast((128, 3, 2, 1)),
                mybir.AluOpType.add,
            )
            nc.vector.tensor_scalar_max(
                stage2[:, :, :, s : s + 1], stage2[:, :, :, s : s + 1], 0.0
            )

        nc.sync.dma_start(out_d[:], stage2[:])

    nc.compile()
    return nc


def _pack_inputs(input, W1, W2, W3, b1, b2, b3):
    # Host-materialized im2col: Xrep[b, k, t] = x[b, t + k//300, k%300],
    # laid out as 12 K-tiles of 128 rows, SEQ padded to 400 with zeros,
    # then regrouped as 4 chunks of 3 K-tiles with the partition dim
    # outermost so each chunk DMAs with 2.4KB contiguous per partition.
    xt = np.zeros((B, EMB, SEQP), np.float32)
    xt[:, :, :SEQ] = np.asarray(input, np.float32).transpose(0, 2, 1)
    xrep = np.zeros((B, KTOT * 128, SEQP), np.float32)
    for j in range(5):
        rows = xrep[:, j * EMB : (j + 1) * EMB, : SEQP - j]
        rows[:] = xt[:, :, j:]
    xt = (
        xrep.reshape(B, NCHUNK, 3, 128, SEQP)
        .transpose(0, 1, 3, 2, 4)
        .astype(np.float16)
    )

    wq = np.zeros((128, 2, NCOL, 128), np.float32)  # cast to fp16 below
    for br, (n, W) in enumerate(zip(NS, (W1, W2, W3))):
        Wt = np.asarray(W, np.float32).T  # [n*300, 256]
        for r in range(NTILES[br]):
            rows = Wt[128 * r : min(128 * (r + 1), n * EMB)]
            for dh in range(2):
                wq[: rows.shape[0], dh, COLB[br] + r, :] = (
                    rows[:, dh * 128 : (dh + 1) * 128]
                )

    wq = wq.astype(np.float16)

    bp = np.empty((128, 3, 2), np.float32)
    for br, b in enumerate((b1, b2, b3)):
        b = np.asarray(b, np.float32).reshape(DEPTH)
        for dh in range(2):
            bp[:, br, dh] = b[dh * 128 : (dh + 1) * 128]
    return xt, wq, bp


def kernel(input, W1, W2, W3, b1, b2, b3):
    global _built, LAST_RESULT
    from concourse.bass_utils import run_bass_kernel_spmd

    xt, wq, bp = _pack_inputs(input, W1, W2, W3, b1, b2, b3)

    if _built is None:
        _built = _build_bass()
    nc = _built

    in_maps = [
        {"xt": xt[c * BPC : (c + 1) * BPC], "wq": wq, "bp": bp}
        for c in range(NCORES)
    ]
    res = run_bass_kernel_spmd(
        nc, in_maps, core_ids=list(range(NCORES)), trace=TRACE
    )
    LAST_RESULT = res

    out = np.empty((B, 3 * DEPTH), np.float32)
    for c in range(NCORES):
        arr = res.results[c]["out_t"]  # [128, 3, 2, BPC]
        out[c * BPC : (c + 1) * BPC] = arr.transpose(3, 1, 2, 0).reshape(BPC, 768)
    return out


# revision 6
# speedup vs baseline: 1.0239x; 1.0239x over previous
"""TextCNN-style conv layer (kernel sizes 3/4/5, EMB=300 -> DEPTH=256, bias,
ReLU, max-pool over time) as a Bass/Tile kernel for 8 Trainium2 NeuronCores.

Strategy: data-parallel over batch (8 samples per core), weights replicated.

Conv as dense-K matmuls over a host-materialized im2col: for branch n, window
output y[d, i] = sum_{k < n*300} Xrep[k, i] * Wn[d, k] with Xrep[k, i] =
x[i + k//300, k%300].  Each branch contracts ceil(n*300/128) dense K=128
tiles (8/10/12 -> 30 matmuls per sample per depth-half); branch boundaries
inside a tile are handled by zero-padding the *weights*.  fp16 operands run
the PE at its full 1 col/cycle rate; warm back-to-back gap is N/2.4GHz+2.5ns,
so the 480-matmul stream floor is ~79.4us.

v2 changes (from trace analysis of the 102.5us baseline):
- DMA: x ships as 4 chunks per sample of [128, 3, 400] fp16 with the DRAM
  laid out partition-major so every chunk is one 307KB transfer with 2.4KB
  contiguous per partition (the old per-K-tile loads were 102KB with 800B
  runs -> only ~230GB/s aggregate and a 2.9us weight stall mid-stream).
  All transfers are issued up-front in global need order, greedily
  byte-balanced across the two HWDGE queues (sync/scalar), which share the
  16 SDMA engines round-robin.
- PE warmup: the HAM clock gate starts every kernel at 1.2GHz and only
  reaches 2.4GHz after ~3.4us of sustained PE activity; the baseline paid
  that on its first 18 real matmuls (~2.9us).  Six dummy N=512 matmuls on a
  memset tile run during the DMA fill window instead.
- Epilogue: bias-add + relu fused per sample right after its reduces, so the
  tail after the last matmul is just one reduce + one tiny DVE op + the 6KB
  output DMA.

Epilogue math: relu(max_i(y + b)) == max(0, max_i y + b): DVE reduce_max over
the window axis straight out of PSUM, output staged [d, branch, half, sample]
per core and de-transposed on host.
"""

import numpy as np

B, SEQ, EMB = 64, 394, 300
DEPTH = 256
NCORES = 8
BPC = B // NCORES  # samples per core
SEQP = 400  # x_t free-dim padded (zeros) so shifted loads stay in bounds
NS = (3, 4, 5)
NTILES = (8, 10, 12)  # ceil(n*300/128) K-tiles per branch
COLB = (0, 8, 18)  # weight column base per branch
NCOL = 30
KTOT = 12  # distinct Xrep K-tiles per sample
NCHUNK = 4  # x DMA granularity: 4 chunks of 3 K-tiles per sample

TRACE = False
LAST_RESULT = None

_built = None


def _build_bass():
    import concourse.mybir as mybir
    import concourse.tile as tile
    from concourse import bacc
    from contextlib import ExitStack

    f32 = mybir.dt.float32
    f16 = mybir.dt.float16

    nc = bacc.Bacc("TRN2", target_bir_lowering=False)
    xt_d = nc.dram_tensor("xt", (BPC, NCHUNK, 128, 3, SEQP), f16, kind="ExternalInput")
    wq_d = nc.dram_tensor("wq", (128, 2, NCOL, 128), f16, kind="ExternalInput")
    bp_d = nc.dram_tensor("bp", (128, 3, 2), f32, kind="ExternalInput")
    out_d = nc.dram_tensor("out_t", (128, 3, 2, BPC), f32, kind="ExternalOutput")

    with tile.TileContext(nc) as tc, ExitStack() as ctx:
        xpool = ctx.enter_context(tc.tile_pool(name="x", bufs=1))
        wpool = ctx.enter_context(tc.tile_pool(name="w", bufs=1))
        cpool = ctx.enter_context(tc.tile_pool(name="consts", bufs=1))
        spool = ctx.enter_context(tc.tile_pool(name="stage", bufs=1))
        pspool = ctx.enter_context(tc.tile_pool(name="ps", bufs=8, space="PSUM"))

        # --- PE warmup: six dummy N=512 matmuls on memset tiles so the HAM
        # clock gate reaches 2.4GHz while the first DMAs are still in flight.
        wz = cpool.tile([128, 128], f16)
        xz = cpool.tile([128, 512], f16)
        dmy = cpool.tile([128, 1], f32)
        nc.vector.memset(wz[:], 0.0)
        nc.vector.memset(xz[:], 0.0)
        psw = pspool.tile([128, 512], f32, tag="ps", name="ps_warm")
        NWARM = 12
        for i in range(NWARM):
            nc.tensor.matmul(
                psw[:], lhsT=wz[:], rhs=xz[:], start=(i == 0), stop=(i == NWARM - 1)
            )
        nc.vector.reduce_max(dmy[:], psw[:], axis=mybir.AxisListType.X)

        # --- SBUF destination tiles
        wts = {}
        for dh in range(2):
            for br in range(3):
                wts[dh, br] = wpool.tile(
                    [128, NTILES[br], 128], f16, tag=f"w{dh}{br}", name=f"w{dh}{br}"
                )
        xs = {}
        for s in range(BPC):
            for c in range(NCHUNK):
                xs[s, c] = xpool.tile(
                    [128, 3, SEQP], f16, tag=f"x{s}c{c}", name=f"x{s}c{c}"
                )
        bt = cpool.tile([128, 3, 2], f32)

        # --- All loads issued up-front in global need order, greedily
        # byte-balanced across the two HWDGE queues (they share the SDMA
        # engines at packet granularity, so each drains ~half the bytes).
        def _w_item(dh, br):
            nt = NTILES[br]
            return (nt * 256 * 128, wts[dh, br][:], wq_d[:, dh, COLB[br] : COLB[br] + nt, :])

        def _x_item(s, c):
            return (3 * SEQP * 2 * 128, xs[s, c][:], xt_d[s, c])

        # Hand-scheduled per-queue load order.  Each queue drains in FIFO at
        # ~160GB/s (they share the 16 SDMA engines) with ~1.8us between last
        # byte and the completion semaphore, so items are laid out to match
        # the matmul stream's consumption order: sample-0 x and the first
        # weight group lead (w00 split so the first matmuls gate on 64KB),
        # later weight groups and the tiny bias slot in where their deadline
        # allows.
        bias_item = (3 * 2 * 4 * 128, bt[:], bp_d[:])
        sync_items = [
            (2 * 256 * 128, wts[0, 0][:, 0:2, :], wq_d[:, 0, 0:2, :]),
            (6 * 256 * 128, wts[0, 0][:, 2:8, :], wq_d[:, 0, 2:8, :]),
            _x_item(0, 1),
            _w_item(0, 1),
            _x_item(0, 3),
            _w_item(1, 1),
            bias_item,
            _x_item(1, 1),
            _x_item(1, 3),
        ]
        scalar_items = [
            (SEQP * 2 * 128, xs[0, 0][:, 0:1, :], xt_d[0, 0, :, 0:1, :]),
            (2 * SEQP * 2 * 128, xs[0, 0][:, 1:3, :], xt_d[0, 0, :, 1:3, :]),
            _x_item(0, 2),
            _w_item(0, 2),
            _w_item(1, 0),
            _w_item(1, 2),
            _x_item(1, 0),
            _x_item(1, 2),
        ]
        for s in range(2, BPC):
            sync_items += [_x_item(s, 1), _x_item(s, 3)]
            scalar_items += [_x_item(s, 0), _x_item(s, 2)]
        for eng, lst in ((nc.sync, sync_items), (nc.scalar, scalar_items)):
            for nbytes, dst, src in lst:
                eng.dma_start(dst, src)

        stage = spool.tile([128, 3, 2, BPC], f32)
        stage2 = spool.tile([128, 3, 2, BPC], f32)

        def do_group(s, dh, br):
            n = NS[br]
            nw = SEQ - n  # windows the reference maxes over
            nmm = nw + (nw & 1)  # keep the moving free-dim count even
            nt = NTILES[br]
            ps = pspool.tile([128, 512], f32, tag="ps", name=f"ps_{s}_{dh}_{br}")
            for r in range(nt):
                kk = min(128, 5 * EMB - 128 * r)  # 92 on the last tile
                nc.tensor.matmul(
                    ps[:, :nmm],
                    lhsT=wts[dh, br][:kk, r, :],
                    rhs=xs[s, r // 3][:kk, r % 3, :nmm],
                    start=(r == 0),
                    stop=(r == nt - 1),
                )
            nc.vector.reduce_max(
                stage[:, br, dh, s : s + 1],
                ps[:, :nw],
                axis=mybir.AxisListType.X,
            )

        for s in range(BPC):
            for dh in range(2):
                for br in range(3):
                    do_group(s, dh, br)
            # fused bias + relu for this sample's column, off the tail
            nc.vector.tensor_tensor(
                stage2[:, :, :, s : s + 1],
                stage[:, :, :, s : s + 1],
                bt[:, :, :, None].to_broadcast((128, 3, 2, 1)),
                mybir.AluOpType.add,
            )
            nc.vector.tensor_scalar_max(
                stage2[:, :, :, s : s + 1], stage2[:, :, :, s : s + 1], 0.0
            )

        nc.sync.dma_start(out_d[:], stage2[:])

    nc.compile()
    return nc


def _pack_inputs(input, W1, W2, W3, b1, b2, b3):
    # Host-materialized im2col: Xrep[b, k, t] = x[b, t + k//300, k%300],
    # laid out as 12 K-tiles of 128 rows, SEQ padded to 400 with zeros,
    # then regrouped as 4 chunks of 3 K-tiles with the partition dim
    # outermost so each chunk DMAs with 2.4KB contiguous per partition.
    xt = np.zeros((B, EMB, SEQP), np.float32)
    xt[:, :, :SEQ] = np.asarray(input, np.float32).transpose(0, 2, 1)
    xrep = np.zeros((B, KTOT * 128, SEQP), np.float32)
    for j in range(5):
        rows = xrep[:, j * EMB : (j + 1) * EMB, : SEQP - j]
        rows[:] = xt[:, :, j:]
    xt = (
        xrep.reshape(B, NCHUNK, 3, 128, SEQP)
        .transpose(0, 1, 3, 2, 4)
        .astype(np.float16)
    )

    wq = np.zeros((128, 2, NCOL, 128), np.float32)  # cast to fp16 below
    for br, (n, W) in enumerate(zip(NS, (W1, W2, W3))):
        Wt = np.asarray(W, np.float32).T  # [n*300, 256]
        for r in range(NTILES[br]):
            rows = Wt[128 * r : min(128 * (r + 1), n * EMB)]
            for dh in range(2):
                wq[: rows.shape[0], dh, COLB[br] + r, :] = (
                    rows[:, dh * 128 : (dh + 1) * 128]
                )

    wq = wq.astype(np.float16)

    bp = np.empty((128, 3, 2), np.float32)
    for br, b in enumerate((b1, b2, b3)):
        b = np.asarray(b, np.float32).reshape(DEPTH)
        for dh in range(2):
            bp[:, br, dh] = b[dh * 128 : (dh + 1) * 128]
    return xt, wq, bp


def kernel(input, W1, W2, W3, b1, b2, b3):
    global _built, LAST_RESULT
    from concourse.bass_utils import run_bass_kernel_spmd

    xt, wq, bp = _pack_inputs(input, W1, W2, W3, b1, b2, b3)

    if _built is None:
        _built = _build_bass()
    nc = _built

    in_maps = [
        {"xt": xt[c * BPC : (c + 1) * BPC], "wq": wq, "bp": bp}
        for c in range(NCORES)
    ]
    res = run_bass_kernel_spmd(
        nc, in_maps, core_ids=list(range(NCORES)), trace=TRACE
    )
    LAST_RESULT = res

    out = np.empty((B, 3 * DEPTH), np.float32)
    for c in range(NCORES):
        arr = res.results[c]["out_t"]  # [128, 3, 2, BPC]
        out[c * BPC : (c + 1) * BPC] = arr.transpose(3, 1, 2, 0).reshape(BPC, 768)
    return out


# revision 22
# speedup vs baseline: 1.0264x; 1.0025x over previous
"""TextCNN-style conv layer (kernel sizes 3/4/5, EMB=300 -> DEPTH=256, bias,
ReLU, max-pool over time) as a Bass/Tile kernel for 8 Trainium2 NeuronCores.

Strategy: data-parallel over batch (8 samples per core), weights replicated.

Conv as dense-K matmuls over a host-materialized im2col: for branch n, window
output y[d, i] = sum_{k < n*300} Xrep[k, i] * Wn[d, k] with Xrep[k, i] =
x[i + k//300, k%300].  Each branch contracts ceil(n*300/128) dense K=128
tiles (8/10/12 -> 30 matmuls per sample per depth-half); branch boundaries
inside a tile are handled by zero-padding the *weights*.  fp16 operands run
the PE at its full 1 col/cycle rate; warm back-to-back gap is N/2.4GHz+2.5ns,
so the 480-matmul stream floor is ~79.4us.

v2 changes (from trace analysis of the 102.5us baseline):
- DMA: x ships as 4 chunks per sample of [128, 3, 400] fp16 with the DRAM
  laid out partition-major so every chunk is one 307KB transfer with 2.4KB
  contiguous per partition (the old per-K-tile loads were 102KB with 800B
  runs -> only ~230GB/s aggregate and a 2.9us weight stall mid-stream).
  All transfers are issued up-front in global need order, greedily
  byte-balanced across the two HWDGE queues (sync/scalar), which share the
  16 SDMA engines round-robin.
- PE warmup: the HAM clock gate starts every kernel at 1.2GHz and only
  reaches 2.4GHz after ~3.4us of sustained PE activity; the baseline paid
  that on its first 18 real matmuls (~2.9us).  Six dummy N=512 matmuls on a
  memset tile run during the DMA fill window instead.
- Epilogue: bias-add + relu fused per sample right after its reduces, so the
  tail after the last matmul is just one reduce + one tiny DVE op + the 6KB
  output DMA.

Epilogue math: relu(max_i(y + b)) == max(0, max_i y + b): DVE reduce_max over
the window axis straight out of PSUM, output staged [d, branch, half, sample]
per core and de-transposed on host.
"""

import numpy as np

B, SEQ, EMB = 64, 394, 300
DEPTH = 256
NCORES = 8
BPC = B // NCORES  # samples per core
SEQP = 400  # x_t free-dim padded (zeros) so shifted loads stay in bounds
NS = (3, 4, 5)
NTILES = (8, 10, 12)  # ceil(n*300/128) K-tiles per branch
NMAIN = (7, 9, 11)  # full-K tiles per branch; the K=4/48/92 tails run as strips
COLB = (0, 8, 18)  # weight column base per branch
NCOL = 30
KTOT = 12  # distinct Xrep K-tiles per sample
NCHUNK = 4  # x DMA granularity: 4 chunks of 3 K-tiles per sample

TRACE = False
LAST_RESULT = None

_built = None


def _build_bass():
    import concourse.mybir as mybir
    import concourse.tile as tile
    from concourse import bacc
    from contextlib import ExitStack

    f32 = mybir.dt.float32
    f16 = mybir.dt.float16

    nc = bacc.Bacc("TRN2", target_bir_lowering=False)
    xt_d = nc.dram_tensor("xt", (BPC, 128, 13, SEQP), f16, kind="ExternalInput")
    wq_d = nc.dram_tensor("wq", (128, 2, NCOL, 128), f16, kind="ExternalInput")
    bp_d = nc.dram_tensor("bp", (128, 3, 2), f32, kind="ExternalInput")
    out_d = nc.dram_tensor("out_t", (128, 3, 2, BPC), f32, kind="ExternalOutput")

    with tile.TileContext(nc) as tc, ExitStack() as ctx:
        xpool = ctx.enter_context(tc.tile_pool(name="x", bufs=1))
        wpool = ctx.enter_context(tc.tile_pool(name="w", bufs=1))
        cpool = ctx.enter_context(tc.tile_pool(name="consts", bufs=1))
        spool = ctx.enter_context(tc.tile_pool(name="stage", bufs=1))
        pspool = ctx.enter_context(tc.tile_pool(name="ps", bufs=8, space="PSUM"))

        # --- PE warmup: six dummy N=512 matmuls on memset tiles so the HAM
        # clock gate reaches 2.4GHz while the first DMAs are still in flight.
        wz = cpool.tile([128, 128], f16)
        xz = cpool.tile([128, 512], f16)
        dmy = cpool.tile([128, 1], f32)
        nc.vector.memset(wz[:], 0.0)
        nc.vector.memset(xz[:], 0.0)
        psw = pspool.tile([128, 512], f32, tag="ps", name="ps_warm")
        NWARM = 12
        for i in range(NWARM):
            nc.tensor.matmul(
                psw[:], lhsT=wz[:], rhs=xz[:], start=(i == 0), stop=(i == NWARM - 1)
            )
        nc.vector.reduce_max(dmy[:], psw[:], axis=mybir.AxisListType.X)

        # --- SBUF destination tiles
        wts = {}
        for dh in range(2):
            for br in range(3):
                wts[dh, br] = wpool.tile(
                    [128, NTILES[br], 128], f16, tag=f"w{dh}{br}", name=f"w{dh}{br}"
                )
        xs = {}
        for s in range(BPC):
            xs[s] = xpool.tile([128, 13, SEQP], f16, tag=f"x{s}", name=f"x{s}")
        bt = cpool.tile([128, 3, 2], f32)

        # --- All loads issued up-front in global need order, greedily
        # byte-balanced across the two HWDGE queues (they share the SDMA
        # engines at packet granularity, so each drains ~half the bytes).
        def _w_item(dh, br):
            nt = NTILES[br]
            return (nt * 256 * 128, wts[dh, br][:], wq_d[:, dh, COLB[br] : COLB[br] + nt, :])

        # x chunk c covers im2col cols [lo, hi) of the per-sample tile
        CHUNKS = ((0, 3), (3, 6), (6, 9), (9, 13))

        def _x_item(s, c):
            lo, hi = CHUNKS[c]
            return (
                (hi - lo) * SEQP * 2 * 128,
                xs[s][:, lo:hi, :],
                xt_d[s, :, lo:hi, :],
            )

        # Hand-scheduled per-queue load order.  Each queue drains in FIFO at
        # ~160GB/s (they share the 16 SDMA engines) with ~1.8us between last
        # byte and the completion semaphore, so items are laid out to match
        # the matmul stream's consumption order: sample-0 x and the first
        # weight group lead (w00 split so the first matmuls gate on 64KB),
        # later weight groups and the tiny bias slot in where their deadline
        # allows.
        bias_item = (3 * 2 * 4 * 128, bt[:], bp_d[:])
        sync_items = [
            (2 * 256 * 128, wts[0, 0][:, 0:2, :], wq_d[:, 0, 0:2, :]),
            (6 * 256 * 128, wts[0, 0][:, 2:8, :], wq_d[:, 0, 2:8, :]),
            _x_item(0, 1),
            _w_item(0, 1),
            _x_item(0, 3),
            _w_item(1, 1),
            bias_item,
            _x_item(1, 1),
            _x_item(1, 3),
        ]
        scalar_items = [
            (SEQP * 2 * 128, xs[0][:, 0:1, :], xt_d[0, :, 0:1, :]),
            (2 * SEQP * 2 * 128, xs[0][:, 1:3, :], xt_d[0, :, 1:3, :]),
            _x_item(0, 2),
            _w_item(0, 2),
            _w_item(1, 0),
            _w_item(1, 2),
            _x_item(1, 0),
            _x_item(1, 2),
        ]
        for s in range(2, BPC):
            sync_items += [_x_item(s, 1), _x_item(s, 3)]
            scalar_items += [_x_item(s, 0), _x_item(s, 2)]
        for eng, lst in ((nc.sync, sync_items), (nc.scalar, scalar_items)):
            for nbytes, dst, src in lst:
                eng.dma_start(dst, src)

        stage = spool.tile([128, 3, 2, BPC], f32)
        stage2 = spool.tile([128, 3, 2, BPC], f32)

        NMMS = tuple((SEQ - n) + ((SEQ - n) & 1) for n in NS)  # even moving count

        def do_half(s, dh):
            for br in range(3):
                nmm = NMMS[br]
                nt = NTILES[br]
                ps = pspool.tile(
                    [128, 512], f32, tag="ps", name=f"ps_{s}_{dh}_{br}"
                )
                for r in range(nt):
                    kk = min(128, 5 * EMB - 128 * r)  # 92 on the last tile
                    nc.tensor.matmul(
                        ps[:, :nmm],
                        lhsT=wts[dh, br][:kk, r, :],
                        rhs=xs[s][:kk, r, :nmm],
                        start=(r == 0),
                        stop=(r == nt - 1),
                    )
                nc.vector.reduce_max(
                    stage[:, br, dh, s : s + 1],
                    ps[:, : SEQ - NS[br]],
                    axis=mybir.AxisListType.X,
                )

        for s in range(BPC):
            for dh in range(2):
                do_half(s, dh)
            # fused bias + relu for this sample's column, off the tail
            nc.vector.tensor_tensor(
                stage2[:, :, :, s : s + 1],
                stage[:, :, :, s : s + 1],
                bt[:, :, :, None].to_broadcast((128, 3, 2, 1)),
                mybir.AluOpType.add,
            )
            nc.vector.tensor_scalar_max(
                stage2[:, :, :, s : s + 1], stage2[:, :, :, s : s + 1], 0.0
            )

        nc.sync.dma_start(out_d[:], stage2[:])

    nc.compile()
    return nc


def _pack_inputs(input, W1, W2, W3, b1, b2, b3):
    # Host-materialized im2col: Xrep[b, k, t] = x[b, t + k//300, k%300],
    # laid out as 12 K-tiles of 128 rows, SEQ padded to 400 with zeros,
    # then regrouped as 4 chunks of 3 K-tiles with the partition dim
    # outermost so each chunk DMAs with 2.4KB contiguous per partition.
    xt = np.zeros((B, EMB, SEQP), np.float32)
    xt[:, :, :SEQ] = np.asarray(input, np.float32).transpose(0, 2, 1)
    xrep = np.zeros((B, KTOT * 128, SEQP), np.float32)
    for j in range(5):
        rows = xrep[:, j * EMB : (j + 1) * EMB, : SEQP - j]
        rows[:] = xt[:, :, j:]
    # Per-sample layout [128, 13, SEQP]: cols 0-11 are the 12 im2col K-tiles
    # (partition-major so chunk DMAs are per-partition contiguous); col 12
    # holds br0's K-tail (im2col rows 896-899) relocated to partitions 64-67
    # so it can run as the row-group-2 strip of batch B.
    xt = np.zeros((B, 128, 13, SEQP), np.float32)
    xt[:, :, :12, :] = xrep.reshape(B, 12, 128, SEQP).transpose(0, 2, 1, 3)
    xt[:, 64:68, 12, :] = xrep[:, 896:900, :]
    xt = xt.astype(np.float16)

    wq = np.zeros((128, 2, NCOL, 128), np.float32)  # cast to fp16 below
    for br, (n, W) in enumerate(zip(NS, (W1, W2, W3))):
        Wt = np.asarray(W, np.float32).T  # [n*300, 256]
        for r in range(NTILES[br]):
            rows = Wt[128 * r : min(128 * (r + 1), n * EMB)]
            for dh in range(2):
                wq[: rows.shape[0], dh, COLB[br] + r, :] = (
                    rows[:, dh * 128 : (dh + 1) * 128]
                )
    wq = wq.astype(np.float16)

    bp = np.empty((128, 3, 2), np.float32)
    for br, b in enumerate((b1, b2, b3)):
        b = np.asarray(b, np.float32).reshape(DEPTH)
        for dh in range(2):
            bp[:, br, dh] = b[dh * 128 : (dh + 1) * 128]
    return xt, wq, bp


def kernel(input, W1, W2, W3, b1, b2, b3):
    global _built, LAST_RESULT
    from concourse.bass_utils import run_bass_kernel_spmd

    xt, wq, bp = _pack_inputs(input, W1, W2, W3, b1, b2, b3)

    if _built is None:
        _built = _build_bass()
    nc = _built

    in_maps = [
        {"xt": xt[c * BPC : (c + 1) * BPC], "wq": wq, "bp": bp}
        for c in range(NCORES)
    ]
    res = run_bass_kernel_spmd(
        nc, in_maps, core_ids=list(range(NCORES)), trace=TRACE
    )
    LAST_RESULT = res

    out = np.empty((B, 3 * DEPTH), np.float32)
    for c in range(NCORES):
        arr = res.results[c]["out_t"]  # [128, 3, 2, BPC]
        out[c * BPC : (c + 1) * BPC] = arr.transpose(3, 1, 2, 0).reshape(BPC, 768)
    return out


# revision 23
# speedup vs baseline: 1.0273x; 1.0009x over previous
"""TextCNN-style conv layer (kernel sizes 3/4/5, EMB=300 -> DEPTH=256, bias,
ReLU, max-pool over time) as a Bass/Tile kernel for 8 Trainium2 NeuronCores.

Strategy: data-parallel over batch (8 samples per core), weights replicated.

Conv as dense-K matmuls over a host-materialized im2col: for branch n, window
output y[d, i] = sum_{k < n*300} Xrep[k, i] * Wn[d, k] with Xrep[k, i] =
x[i + k//300, k%300].  Each branch contracts ceil(n*300/128) dense K=128
tiles (8/10/12 -> 30 matmuls per sample per depth-half); branch boundaries
inside a tile are handled by zero-padding the *weights*.  fp16 operands run
the PE at its full 1 col/cycle rate; warm back-to-back gap is N/2.4GHz+2.5ns,
so the 480-matmul stream floor is ~79.4us.

Changes from the 102.5us baseline (each driven by NTFF trace analysis):
- DMA: per sample the im2col lives in one [128, 13, 400] fp16 DRAM block
  (partition-major; col 12 spare), shipped as 4 chunk transfers of ~300KB
  with 2.4KB contiguous per partition (the old per-K-tile loads were 102KB
  with 800B runs -> only ~230GB/s aggregate and a 2.9us weight stall
  mid-stream; big chunks measure ~375GB/s).  Transfers are hand-ordered per
  HWDGE queue (sync/scalar share the 16 SDMA engines round-robin, each
  queue drains FIFO with ~1.8us completion lag): sample-0 x and the first
  weight group lead, w00 split so the first matmuls gate on 64KB, the tiny
  bias transfer demoted behind sample 1 (its 24B-descriptor slow path once
  delayed the whole scalar queue by 2.5us).
- PE warmup: the HAM clock gate starts every kernel at 1.2GHz and reaches
  2.4GHz only after ~3.4us of *uninterrupted* PE activity; any stall resets
  the window (the baseline paid half-clock on its first ~18 matmuls, and a
  v3 of this kernel with early DMA stalls stayed cold to t=20us).  Twelve
  dummy N=512 matmuls on a memset tile bridge the DMA fill window so the
  real stream starts warm with its inputs resident.
- Epilogue: bias-add + relu fused per sample right after its reduces, so
  the tail after the last matmul is one reduce + one tiny DVE op + the 6KB
  output DMA.
(Attempted and reverted: packing the K=92/48/4 branch tails as concurrent
32-row-group strip matmuls via tile_position — would save ~2.5us of PE
time but hangs the device on both row-group-3-explicit and auto variants.)

Epilogue math: relu(max_i(y + b)) == max(0, max_i y + b): DVE reduce_max over
the window axis straight out of PSUM, output staged [d, branch, half, sample]
per core and de-transposed on host.
"""

import numpy as np

B, SEQ, EMB = 64, 394, 300
DEPTH = 256
NCORES = 8
BPC = B // NCORES  # samples per core
SEQP = 400  # x_t free-dim padded (zeros) so shifted loads stay in bounds
NS = (3, 4, 5)
NTILES = (8, 10, 12)  # ceil(n*300/128) K-tiles per branch
NMAIN = (7, 9, 11)  # full-K tiles per branch; the K=4/48/92 tails run as strips
COLB = (0, 8, 18)  # weight column base per branch
NCOL = 30
KTOT = 12  # distinct Xrep K-tiles per sample
NCHUNK = 4  # x DMA granularity: 4 chunks of 3 K-tiles per sample

TRACE = False
LAST_RESULT = None

_built = None


def _build_bass():
    import concourse.mybir as mybir
    import concourse.tile as tile
    from concourse import bacc
    from contextlib import ExitStack

    f32 = mybir.dt.float32
    f16 = mybir.dt.float16

    nc = bacc.Bacc("TRN2", target_bir_lowering=False)
    xt_d = nc.dram_tensor("xt", (BPC, 128, 13, SEQP), f16, kind="ExternalInput")
    wq_d = nc.dram_tensor("wq", (128, 2, NCOL, 128), f16, kind="ExternalInput")
    bp_d = nc.dram_tensor("bp", (128, 3, 2), f32, kind="ExternalInput")
    out_d = nc.dram_tensor("out_t", (128, 3, 2, BPC), f32, kind="ExternalOutput")

    with tile.TileContext(nc) as tc, ExitStack() as ctx:
        xpool = ctx.enter_context(tc.tile_pool(name="x", bufs=1))
        wpool = ctx.enter_context(tc.tile_pool(name="w", bufs=1))
        cpool = ctx.enter_context(tc.tile_pool(name="consts", bufs=1))
        spool = ctx.enter_context(tc.tile_pool(name="stage", bufs=1))
        pspool = ctx.enter_context(tc.tile_pool(name="ps", bufs=8, space="PSUM"))

        # --- PE warmup: six dummy N=512 matmuls on memset tiles so the HAM
        # clock gate reaches 2.4GHz while the first DMAs are still in flight.
        wz = cpool.tile([128, 128], f16)
        xz = cpool.tile([128, 512], f16)
        dmy = cpool.tile([128, 1], f32)
        nc.vector.memset(wz[:], 0.0)
        nc.vector.memset(xz[:], 0.0)
        psw = pspool.tile([128, 512], f32, tag="ps", name="ps_warm")
        NWARM = 12
        for i in range(NWARM):
            nc.tensor.matmul(
                psw[:], lhsT=wz[:], rhs=xz[:], start=(i == 0), stop=(i == NWARM - 1)
            )
        nc.vector.reduce_max(dmy[:], psw[:], axis=mybir.AxisListType.X)

        # --- SBUF destination tiles
        wts = {}
        for dh in range(2):
            for br in range(3):
                wts[dh, br] = wpool.tile(
                    [128, NTILES[br], 128], f16, tag=f"w{dh}{br}", name=f"w{dh}{br}"
                )
        xs = {}
        for s in range(BPC):
            xs[s] = xpool.tile([128, 13, SEQP], f16, tag=f"x{s}", name=f"x{s}")
        bt = cpool.tile([128, 3, 2], f32)

        # --- All loads issued up-front in global need order, greedily
        # byte-balanced across the two HWDGE queues (they share the SDMA
        # engines at packet granularity, so each drains ~half the bytes).
        def _w_item(dh, br):
            nt = NTILES[br]
            return (nt * 256 * 128, wts[dh, br][:], wq_d[:, dh, COLB[br] : COLB[br] + nt, :])

        # x chunk c covers im2col cols [lo, hi) of the per-sample tile
        CHUNKS = ((0, 3), (3, 6), (6, 9), (9, 13))

        def _x_item(s, c):
            lo, hi = CHUNKS[c]
            return (
                (hi - lo) * SEQP * 2 * 128,
                xs[s][:, lo:hi, :],
                xt_d[s, :, lo:hi, :],
            )

        # Hand-scheduled per-queue load order.  Each queue drains in FIFO at
        # ~160GB/s (they share the 16 SDMA engines) with ~1.8us between last
        # byte and the completion semaphore, so items are laid out to match
        # the matmul stream's consumption order: sample-0 x and the first
        # weight group lead (w00 split so the first matmuls gate on 64KB),
        # later weight groups and the tiny bias slot in where their deadline
        # allows.
        bias_item = (3 * 2 * 4 * 128, bt[:], bp_d[:])
        sync_items = [
            (2 * 256 * 128, wts[0, 0][:, 0:2, :], wq_d[:, 0, 0:2, :]),
            (6 * 256 * 128, wts[0, 0][:, 2:8, :], wq_d[:, 0, 2:8, :]),
            _x_item(0, 1),
            _w_item(0, 1),
            _x_item(0, 3),
            _w_item(1, 1),
            bias_item,
            _x_item(1, 1),
            _x_item(1, 3),
        ]
        scalar_items = [
            (SEQP * 2 * 128, xs[0][:, 0:1, :], xt_d[0, :, 0:1, :]),
            (2 * SEQP * 2 * 128, xs[0][:, 1:3, :], xt_d[0, :, 1:3, :]),
            _x_item(0, 2),
            _w_item(0, 2),
            _w_item(1, 0),
            _w_item(1, 2),
            _x_item(1, 0),
            _x_item(1, 2),
        ]
        for s in range(2, BPC):
            sync_items += [_x_item(s, 1), _x_item(s, 3)]
            scalar_items += [_x_item(s, 0), _x_item(s, 2)]
        for eng, lst in ((nc.sync, sync_items), (nc.scalar, scalar_items)):
            for nbytes, dst, src in lst:
                eng.dma_start(dst, src)

        stage = spool.tile([128, 3, 2, BPC], f32)
        stage2 = spool.tile([128, 3, 2, BPC], f32)

        NMMS = tuple((SEQ - n) + ((SEQ - n) & 1) for n in NS)  # even moving count

        def do_half(s, dh):
            for br in range(3):
                nmm = NMMS[br]
                nt = NTILES[br]
                ps = pspool.tile(
                    [128, 512], f32, tag="ps", name=f"ps_{s}_{dh}_{br}"
                )
                for r in range(nt):
                    kk = min(128, 5 * EMB - 128 * r)  # 92 on the last tile
                    nc.tensor.matmul(
                        ps[:, :nmm],
                        lhsT=wts[dh, br][:kk, r, :],
                        rhs=xs[s][:kk, r, :nmm],
                        start=(r == 0),
                        stop=(r == nt - 1),
                    )
                nc.vector.reduce_max(
                    stage[:, br, dh, s : s + 1],
                    ps[:, : SEQ - NS[br]],
                    axis=mybir.AxisListType.X,
                )

        for s in range(BPC):
            for dh in range(2):
                do_half(s, dh)
            # fused bias + relu for this sample's column, off the tail
            nc.vector.tensor_tensor(
                stage2[:, :, :, s : s + 1],
                stage[:, :, :, s : s + 1],
                bt[:, :, :, None].to_broadcast((128, 3, 2, 1)),
                mybir.AluOpType.add,
            )
            nc.vector.tensor_scalar_max(
                stage2[:, :, :, s : s + 1], stage2[:, :, :, s : s + 1], 0.0
            )

        nc.sync.dma_start(out_d[:], stage2[:])

    nc.compile()
    return nc


def _pack_inputs(input, W1, W2, W3, b1, b2, b3):
    # Host-materialized im2col: Xrep[b, k, t] = x[b, t + k//300, k%300],
    # laid out as 12 K-tiles of 128 rows, SEQ padded to 400 with zeros,
    # then regrouped as 4 chunks of 3 K-tiles with the partition dim
    # outermost so each chunk DMAs with 2.4KB contiguous per partition.
    xt = np.zeros((B, EMB, SEQP), np.float32)
    xt[:, :, :SEQ] = np.asarray(input, np.float32).transpose(0, 2, 1)
    xrep = np.zeros((B, KTOT * 128, SEQP), np.float32)
    for j in range(5):
        rows = xrep[:, j * EMB : (j + 1) * EMB, : SEQP - j]
        rows[:] = xt[:, :, j:]
    # Per-sample layout [128, 13, SEQP]: cols 0-11 are the 12 im2col K-tiles
    # (partition-major so chunk DMAs are per-partition contiguous); col 12
    # holds br0's K-tail (im2col rows 896-899) relocated to partitions 64-67
    # so it can run as the row-group-2 strip of batch B.
    xt = np.zeros((B, 128, 13, SEQP), np.float32)
    xt[:, :, :12, :] = xrep.reshape(B, 12, 128, SEQP).transpose(0, 2, 1, 3)
    xt[:, 64:68, 12, :] = xrep[:, 896:900, :]
    xt = xt.astype(np.float16)

    wq = np.zeros((128, 2, NCOL, 128), np.float32)  # cast to fp16 below
    for br, (n, W) in enumerate(zip(NS, (W1, W2, W3))):
        Wt = np.asarray(W, np.float32).T  # [n*300, 256]
        for r in range(NTILES[br]):
            rows = Wt[128 * r : min(128 * (r + 1), n * EMB)]
            for dh in range(2):
                wq[: rows.shape[0], dh, COLB[br] + r, :] = (
                    rows[:, dh * 128 : (dh + 1) * 128]
                )
    wq = wq.astype(np.float16)

    bp = np.empty((128, 3, 2), np.float32)
    for br, b in enumerate((b1, b2, b3)):
        b = np.asarray(b, np.float32).reshape(DEPTH)
        for dh in range(2):
            bp[:, br, dh] = b[dh * 128 : (dh + 1) * 128]
    return xt, wq, bp


def kernel(input, W1, W2, W3, b1, b2, b3):
    global _built, LAST_RESULT
    from concourse.bass_utils import run_bass_kernel_spmd

    xt, wq, bp = _pack_inputs(input, W1, W2, W3, b1, b2, b3)

    if _built is None:
        _built = _build_bass()
    nc = _built

    in_maps = [
        {"xt": xt[c * BPC : (c + 1) * BPC], "wq": wq, "bp": bp}
        for c in range(NCORES)
    ]
    res = run_bass_kernel_spmd(
        nc, in_maps, core_ids=list(range(NCORES)), trace=TRACE
    )
    LAST_RESULT = res

    out = np.empty((B, 3 * DEPTH), np.float32)
    for c in range(NCORES):
        arr = res.results[c]["out_t"]  # [128, 3, 2, BPC]
        out[c * BPC : (c + 1) * BPC] = arr.transpose(3, 1, 2, 0).reshape(BPC, 768)
    return out
